# revision 50
# baseline (speedup 1.0000x reference)
"""AdaptiveEMA Trainium2 kernel: out[b,f,s] = ws[s]/tc[s] where
ws = ws*decay + x, tc = tc*decay + 1, decay = exp(dt_norm^power * log_alpha).

Sharding: pure data parallelism over batch B=8 -> one batch per NeuronCore.
Per core: [F=256, S=8192] rows; features map to partitions (2 blocks of 128),
the scan runs along the free (seq) axis via the VectorE tensor_tensor_scan
instruction (the only engine with a first-order-recurrence op, ~2.1 ns/elem).

Engine split per seq chunk (software-pipelined, one round lookahead):
  Sync   : HWDGE DMAs (t with 1-col overlap, x, out)
  GPSIMD : dt = diff(t), and half of the final multiplies
  ScalarE: decay = exp(log_alpha*(dt*inv_mean)^power) as ln/exp/exp with
           per-partition affine scales, and 1/tc = exp(-ln(tc)) — all five
           transcendentals live in ONE activation-table set (ln+exp), so the
           table is loaded exactly once
  VectorE: both scans (ws = ws*d + x, tc = tc*d + 1, carried across chunks
           via initial=prev[:, -1:]), and the other half of the multiplies
The decay tile lives in PSUM: the scans read it through the PSUM port, which
keeps GPSIMD (which shares SBUF ports with the DVE) from degrading the scan
throughput by ~1.7x. Variable chunk sizes shorten pipeline ramp and tail.

Note on EPS: the reference adds 1e-10 to dt_count (65536.0f) and to tc (>=1).
In float32 both additions are exact no-ops, so they are omitted here.
The reference emits NaNs where jax's parallel-prefix f32 cumsum makes diff(t)
slightly negative; this kernel reproduces the identical NaN set (ln(neg)=NaN
propagates through the scan exactly like NaN**0.5 does in the reference).
"""

import numpy as np

B, F, S = 8, 256, 8192
NCORES = 8
# Variable seq chunking: small first chunk primes the serial carry chain
# quickly (pipeline ramp), small last chunk shortens the drain tail.
CHUNK_SIZES = [256, 512, 1024, 2048, 2048, 2048, 256]
assert sum(CHUNK_SIZES) == S
CMAX = max(CHUNK_SIZES)
K = len(CHUNK_SIZES)
PB = 128            # partition block
NFB = F // PB       # feature blocks per core
LN2 = 0.6931471805599453

_CACHED_NC = None


def _patch_tile_drain(tile_mod):
    """This walrus build rejects >1 sem wait on the kernel-tail Drain
    (CTRL_NO_STRUCT: 'Too many sync wait commands'). Re-emit the tail waits
    as standalone single-wait instructions before a waitless drain."""
    if getattr(tile_mod.TileContext, "_drain_patched", False):
        return
    from concourse.vector_clock import ScopedClock

    def _drain_and_barrier(self, tick_clock, wait_clock):
        nc = self.nc
        absorb = nc.sync.nop(nofuse=True, hint="tail_wait_absorb")
        wait_clock.add_sem_waits(
            absorb.ins, ScopedClock({None: tick_clock.global_clock})
        )
        waits = list(absorb.ins.sync_info.on_wait)
        absorb.ins.sync_info.on_wait = waits[:1]
        by_num = {h.num: h for h in self.sems.allocated().values()}
        for w in waits[1:]:
            nc.sync.wait_ge(by_num[w.id], w.wait_value)
        nc.sync.drain()
        nc.all_engine_barrier()
        assert self.sems is not None
        popped = nc._tile_sem_poison_stack.pop()
        assert popped is self._sem_poison
        nc.clear_and_free_semaphores(list(self.sems.allocated().values()))
        # no second all-engine barrier: engines may halt in any order after
        # the clear; the runtime already waits for every engine to finish
        # before the next execution of the NEFF

    tile_mod.TileContext._drain_and_barrier = _drain_and_barrier
    tile_mod.TileContext._drain_patched = True


def _split_multi_waits(nc, mybir):
    """This walrus build encodes at most ONE sync wait per instruction
    ('Too many sync wait commands'). Hoist extra waits onto standalone
    single-wait NoOps on the same engine, directly before the instruction."""
    idx = 0
    for fn in nc.m.functions:
        for blk in fn.blocks:
            new_insts = []
            changed = False
            for ins in blk.instructions:
                si = ins.sync_info
                waits = list(si.on_wait) if si is not None else []
                if len(waits) > 1:
                    changed = True
                    for w in waits[:-1]:
                        idx += 1
                        nop = mybir.InstNoOp(
                            name=f"wait-split-{idx}",
                            engine=ins.engine,
                            ins=[],
                            outs=[],
                            sync_info=mybir.SyncInfo(on_update=[], on_wait=[w]),
                            text_hint="wait_split",
                        )
                        new_insts.append(nop)
                    si.on_wait = [waits[-1]]
                new_insts.append(ins)
            if changed:
                blk.instructions = new_insts


def _build_nc():
    import concourse.bass as bass
    import concourse.tile as tile
    from concourse import mybir

    _patch_tile_drain(tile)
    f32 = mybir.dt.float32
    Op = mybir.AluOpType
    Act = mybir.ActivationFunctionType

    nc = bass.Bass()
    x_d = nc.declare_dram_parameter("x", [F, S], f32, isOutput=False)
    t_d = nc.declare_dram_parameter("t", [F, S], f32, isOutput=False)
    # packed per-feature constants, precomputed on host (O(F) scalar prep):
    # col 0 = log_alpha = -ln2*exp(-log_halflife), col 1 = clip(power),
    # col 2 = inv_mean = dt_count/running_sum_dt
    prm_d = nc.declare_dram_parameter("params", [F, 4], f32, isOutput=False)
    out_d = nc.declare_dram_parameter("out", [F, S], f32, isOutput=True)

    with tile.TileContext(nc) as tc:
        with (
            tc.tile_pool(name="const", bufs=1) as cpool,
            tc.tile_pool(name="tt", bufs=5) as ttp,
            tc.tile_pool(name="tx", bufs=3) as txp,
            tc.tile_pool(name="dt", bufs=3) as dtp,
            tc.tile_pool(name="dps", bufs=2, space="PSUM") as dpsp,
            tc.tile_pool(name="lr", bufs=3) as lrp,
            tc.tile_pool(name="op", bufs=3) as opool,
            tc.tile_pool(name="scan", bufs=3) as scanp,
        ):
            ones = cpool.tile([PB, CMAX], f32, tag="ones")
            nc.gpsimd.memset(ones[:], 1.0)

            # Per-feature constants [128,1] per feature block, from the
            # host-packed params tensor.
            la, pc, im = [], [], []

            def emit_params():
                for fb in range(NFB):
                    r0 = fb * PB
                    prm = cpool.tile([PB, 4], f32, tag=f"prm{fb}")
                    # issue on the ACT HWDGE queue: lands before the chunk-0
                    # loads finish on the Sync queue, unblocking the first ln
                    nc.scalar.dma_start(prm[:], prm_d[r0 : r0 + PB, 0:4])
                    la.append(prm[:, 0:1])
                    pc.append(prm[:, 1:2])
                    im.append(prm[:, 2:3])

            ws_prev = [None] * NFB
            tcs_prev = [None] * NFB
            dts = {}
            txs = {}

            starts = [sum(CHUNK_SIZES[:j]) for j in range(K)]

            tts = {}

            def emit_load_t(k):
                # t loads run two rounds ahead of use: t heads the long
                # diff -> ACT -> scan chain, and pulling its DMA traffic into
                # the DMA-light early window flattens the mid-kernel peak
                c0, cs = starts[k], CHUNK_SIZES[k]
                for fb in range(NFB):
                    r0 = fb * PB
                    tt = ttp.tile([PB, cs + 1], f32, tag="tt")
                    if k == 0:
                        nc.sync.dma_start(tt[:, 1 : cs + 1], t_d[r0 : r0 + PB, 0:cs])
                    else:
                        nc.sync.dma_start(
                            tt[:, 0 : cs + 1], t_d[r0 : r0 + PB, c0 - 1 : c0 + cs]
                        )
                    tts[(k, fb)] = tt

            def emit_load_x(k):
                c0, cs = starts[k], CHUNK_SIZES[k]
                for fb in range(NFB):
                    r0 = fb * PB
                    tx = txp.tile([PB, cs], f32, tag="tx")
                    nc.sync.dma_start(tx[:], x_d[r0 : r0 + PB, c0 : c0 + cs])
                    txs[(k, fb)] = tx

            def emit_decay(k, fbs=None):
                """diff on GPSIMD, then the 3-op ACT decay chain
                (dt -> ln -> exp -> exp = decay) written into PSUM."""
                c0, cs = starts[k], CHUNK_SIZES[k]
                for fb in (range(NFB) if fbs is None else fbs):
                    r0 = fb * PB
                    tt = tts.pop((k, fb))
                    dt = dtp.tile([PB, cs], f32, tag="dt")
                    if k == 0:
                        nc.gpsimd.tensor_tensor(
                            dt[:, 1:cs], tt[:, 2 : cs + 1], tt[:, 1:cs], op=Op.subtract
                        )
                        nc.gpsimd.memset(dt[:, 0:1], 1.0)
                    else:
                        nc.gpsimd.tensor_tensor(
                            dt[:, 0:cs], tt[:, 1 : cs + 1], tt[:, 0:cs], op=Op.subtract
                        )
                    # decay = exp(log_alpha * (dt*inv_mean)^power). Computed
                    # into PSUM: the scans then read data0 via the PSUM port,
                    # freeing an SBUF read port (GPSIMD shares SBUF ports with
                    # the DVE and was slowing the scans down by ~1.7x).
                    d = dpsp.tile([PB, cs], f32, tag="dps")
                    nc.scalar.activation(d[:], dt[:], Act.Ln, scale=im[fb][:])
                    nc.scalar.activation(d[:], d[:], Act.Exp, scale=pc[fb][:])
                    nc.scalar.activation(d[:], d[:], Act.Exp, scale=la[fb][:])
                    dts[(k, fb)] = d

            def emit_scan_and_out(k, fbs=None):
                c0, cs = starts[k], CHUNK_SIZES[k]
                for fb in (range(NFB) if fbs is None else fbs):
                    r0 = fb * PB
                    d = dts.pop((k, fb))
                    tx = txs.pop((k, fb))

                    # tc scan first: its consumer chain (ln/exp/mult) is longer
                    tcs = scanp.tile([PB, cs], f32, tag="tc")
                    init_tc = 0.0 if k == 0 else tcs_prev[fb]
                    nc.vector.tensor_tensor_scan(
                        tcs[:], d[:], ones[:, 0:cs], init_tc, op0=Op.mult, op1=Op.add
                    )
                    ws = scanp.tile([PB, cs], f32, tag="ws")
                    init_ws = 0.0 if k == 0 else ws_prev[fb]
                    nc.vector.tensor_tensor_scan(
                        ws[:], d[:], tx[:], init_ws, op0=Op.mult, op1=Op.add
                    )
                    ws_prev[fb] = ws[:, cs - 1 : cs]
                    tcs_prev[fb] = tcs[:, cs - 1 : cs]

                    # out = ws / tc via 1/tc = exp(-ln(tc)); tc >= 1, and both
                    # Ln and Exp live in the same ACT table set (no reloads)
                    L = lrp.tile([PB, cs], f32, tag="lr")
                    nc.scalar.activation(L[:], tcs[:], Act.Ln)
                    nc.scalar.activation(L[:], L[:], Act.Exp, scale=-1.0)
                    o = opool.tile([PB, cs], f32, tag="o")
                    # fb0 multiplies on DVE, fb1 on GPSIMD (load balance); the
                    # final round goes all-DVE to shorten the kernel tail
                    if fb == 0 or k == K - 1:
                        nc.vector.tensor_tensor(o[:], ws[:], L[:], op=Op.mult)
                    else:
                        nc.gpsimd.tensor_tensor(o[:], ws[:], L[:], op=Op.mult)
                    nc.sync.dma_start(out_d[r0 : r0 + PB, c0 : c0 + cs], o[:])

            emit_load_t(0)
            emit_load_x(0)
            emit_load_t(1)
            emit_params()
            emit_decay(0)
            for k in range(K):
                # per-fb interleave: the in-order ACT engine gets round-k
                # division work between the DMA/diff-gated round-k+1 decay
                # chains instead of stalling on them
                if k + 1 < K:
                    emit_load_x(k + 1)
                    emit_decay(k + 1, fbs=[0])
                    emit_scan_and_out(k, fbs=[0])
                    emit_decay(k + 1, fbs=[1])
                    emit_scan_and_out(k, fbs=[1])
                    if k + 2 < K:
                        # t prefetch rides at the END of the round's DMA queue
                        # so a tile-buffer stall cannot delay urgent transfers
                        emit_load_t(k + 2)
                else:
                    emit_scan_and_out(k)

    _split_multi_waits(nc, mybir)
    return nc


# Set by a profiling harness (test.py) to capture an NTFF profile of the run.
PROFILE_DIR = None
PROFILE_CORES = [0]


def kernel(x, t, log_halflife, power, running_sum_dt, dt_count):
    global _CACHED_NC
    from concourse.bass_utils import run_bass_kernel_spmd

    if _CACHED_NC is None:
        _CACHED_NC = _build_nc()
    nc = _CACHED_NC

    x = np.ascontiguousarray(np.asarray(x, dtype=np.float32))
    t = np.ascontiguousarray(np.asarray(t, dtype=np.float32))
    # O(F) parameter prep (mirrors the reference's scalar param math)
    lh = np.asarray(log_halflife, dtype=np.float32).reshape(F)
    pw = np.asarray(power, dtype=np.float32).reshape(F)
    rs = np.asarray(running_sum_dt, dtype=np.float32).reshape(F)
    cnt = np.float32(np.asarray(dt_count))
    la = (-np.float32(LN2) * np.exp(-lh)).astype(np.float32)
    pc = np.clip(pw, np.float32(1e-3), np.float32(1.0 - 1e-3)).astype(np.float32)
    im = (cnt / rs).astype(np.float32)
    prm = np.zeros((F, 4), dtype=np.float32)
    prm[:, 0], prm[:, 1], prm[:, 2] = la, pc, im
    prm = np.ascontiguousarray(prm)

    in_maps = [{"x": x[b], "t": t[b], "params": prm} for b in range(NCORES)]

    if PROFILE_DIR is not None:
        import os
        from trn_agent_boot.trn_boot import _ntff_profile_via_ctypes

        os.makedirs(PROFILE_DIR, exist_ok=True)
        hook = _ntff_profile_via_ctypes("/opt/axon/libaxon_pjrt.so")
        with hook(PROFILE_DIR, list(PROFILE_CORES)):
            res = run_bass_kernel_spmd(nc, in_maps, list(range(NCORES)))
    else:
        res = run_bass_kernel_spmd(nc, in_maps, list(range(NCORES)))

    out = np.stack([res.results[b]["out"] for b in range(NCORES)], axis=0)
    return out.astype(np.float32, copy=False)



# revision 51
# speedup vs baseline: 1.0390x; 1.0390x over previous
"""AdaptiveEMA Trainium2 kernel: out[b,f,s] = ws[s]/tc[s] where
ws = ws*decay + x, tc = tc*decay + 1, decay = exp(dt_norm^power * log_alpha).

Sharding: pure data parallelism over batch B=8 -> one batch per NeuronCore.
Per core: [F=256, S=8192] rows; features map to partitions (2 blocks of 128),
the scan runs along the free (seq) axis via the VectorE tensor_tensor_scan
instruction (the only engine with a first-order-recurrence op, ~2.1 ns/elem).

Engine split per seq chunk (software-pipelined, one round lookahead):
  Sync   : HWDGE DMAs (t with 1-col overlap, x, out)
  GPSIMD : dt = diff(t), and half of the final multiplies
  ScalarE: decay = exp(log_alpha*(dt*inv_mean)^power) as ln/exp/exp with
           per-partition affine scales, and 1/tc = exp(-ln(tc)) — all five
           transcendentals live in ONE activation-table set (ln+exp), so the
           table is loaded exactly once
  VectorE: both scans (ws = ws*d + x, tc = tc*d + 1, carried across chunks
           via initial=prev[:, -1:]), and the other half of the multiplies
The decay tile lives in PSUM: the scans read it through the PSUM port, which
keeps GPSIMD (which shares SBUF ports with the DVE) from degrading the scan
throughput by ~1.7x. Variable chunk sizes shorten pipeline ramp and tail.

Note on EPS: the reference adds 1e-10 to dt_count (65536.0f) and to tc (>=1).
In float32 both additions are exact no-ops, so they are omitted here.
The reference emits NaNs where jax's parallel-prefix f32 cumsum makes diff(t)
slightly negative; this kernel reproduces the identical NaN set (ln(neg)=NaN
propagates through the scan exactly like NaN**0.5 does in the reference).
"""

import numpy as np

B, F, S = 8, 256, 8192
NCORES = 8
# Variable seq chunking: small first chunk primes the serial carry chain
# quickly (pipeline ramp), small last chunk shortens the drain tail.
CHUNK_SIZES = [256, 512, 1024, 2048, 2048, 1792, 512]
assert sum(CHUNK_SIZES) == S
CMAX = max(CHUNK_SIZES)
K = len(CHUNK_SIZES)
PB = 128            # partition block
NFB = F // PB       # feature blocks per core
LN2 = 0.6931471805599453

_CACHED_NC = None


def _patch_tile_drain(tile_mod):
    """This walrus build rejects >1 sem wait on the kernel-tail Drain
    (CTRL_NO_STRUCT: 'Too many sync wait commands'). Re-emit the tail waits
    as standalone single-wait instructions before a waitless drain."""
    if getattr(tile_mod.TileContext, "_drain_patched", False):
        return
    from concourse.vector_clock import ScopedClock

    def _drain_and_barrier(self, tick_clock, wait_clock):
        nc = self.nc
        absorb = nc.sync.nop(nofuse=True, hint="tail_wait_absorb")
        wait_clock.add_sem_waits(
            absorb.ins, ScopedClock({None: tick_clock.global_clock})
        )
        waits = list(absorb.ins.sync_info.on_wait)
        absorb.ins.sync_info.on_wait = waits[:1]
        by_num = {h.num: h for h in self.sems.allocated().values()}
        for w in waits[1:]:
            nc.sync.wait_ge(by_num[w.id], w.wait_value)
        nc.sync.drain()
        nc.all_engine_barrier()
        assert self.sems is not None
        popped = nc._tile_sem_poison_stack.pop()
        assert popped is self._sem_poison
        nc.clear_and_free_semaphores(list(self.sems.allocated().values()))
        # no second all-engine barrier: engines may halt in any order after
        # the clear; the runtime already waits for every engine to finish
        # before the next execution of the NEFF

    tile_mod.TileContext._drain_and_barrier = _drain_and_barrier
    tile_mod.TileContext._drain_patched = True


def _split_multi_waits(nc, mybir):
    """This walrus build encodes at most ONE sync wait per instruction
    ('Too many sync wait commands'). Hoist extra waits onto standalone
    single-wait NoOps on the same engine, directly before the instruction."""
    idx = 0
    for fn in nc.m.functions:
        for blk in fn.blocks:
            new_insts = []
            changed = False
            for ins in blk.instructions:
                si = ins.sync_info
                waits = list(si.on_wait) if si is not None else []
                if len(waits) > 1:
                    changed = True
                    for w in waits[:-1]:
                        idx += 1
                        nop = mybir.InstNoOp(
                            name=f"wait-split-{idx}",
                            engine=ins.engine,
                            ins=[],
                            outs=[],
                            sync_info=mybir.SyncInfo(on_update=[], on_wait=[w]),
                            text_hint="wait_split",
                        )
                        new_insts.append(nop)
                    si.on_wait = [waits[-1]]
                new_insts.append(ins)
            if changed:
                blk.instructions = new_insts


def _build_nc():
    import concourse.bass as bass
    import concourse.tile as tile
    from concourse import mybir

    _patch_tile_drain(tile)
    f32 = mybir.dt.float32
    Op = mybir.AluOpType
    Act = mybir.ActivationFunctionType

    nc = bass.Bass()
    x_d = nc.declare_dram_parameter("x", [F, S], f32, isOutput=False)
    t_d = nc.declare_dram_parameter("t", [F, S], f32, isOutput=False)
    # packed per-feature constants, precomputed on host (O(F) scalar prep):
    # col 0 = log_alpha = -ln2*exp(-log_halflife), col 1 = clip(power),
    # col 2 = inv_mean = dt_count/running_sum_dt
    prm_d = nc.declare_dram_parameter("params", [F, 4], f32, isOutput=False)
    out_d = nc.declare_dram_parameter("out", [F, S], f32, isOutput=True)

    with tile.TileContext(nc) as tc:
        with (
            tc.tile_pool(name="const", bufs=1) as cpool,
            tc.tile_pool(name="tt", bufs=5) as ttp,
            tc.tile_pool(name="tx", bufs=3) as txp,
            tc.tile_pool(name="dt", bufs=3) as dtp,
            tc.tile_pool(name="dps", bufs=2, space="PSUM") as dpsp,
            tc.tile_pool(name="lr", bufs=3) as lrp,
            tc.tile_pool(name="op", bufs=3) as opool,
            tc.tile_pool(name="scan", bufs=3) as scanp,
        ):
            ones = cpool.tile([PB, CMAX], f32, tag="ones")
            nc.gpsimd.memset(ones[:], 1.0)

            # Per-feature constants [128,1] per feature block, from the
            # host-packed params tensor.
            la, pc, im = [], [], []

            def emit_params():
                for fb in range(NFB):
                    r0 = fb * PB
                    prm = cpool.tile([PB, 4], f32, tag=f"prm{fb}")
                    # issue on the ACT HWDGE queue: lands before the chunk-0
                    # loads finish on the Sync queue, unblocking the first ln
                    nc.scalar.dma_start(prm[:], prm_d[r0 : r0 + PB, 0:4])
                    la.append(prm[:, 0:1])
                    pc.append(prm[:, 1:2])
                    im.append(prm[:, 2:3])

            ws_prev = [None] * NFB
            tcs_prev = [None] * NFB
            dts = {}
            txs = {}

            starts = [sum(CHUNK_SIZES[:j]) for j in range(K)]

            tts = {}

            def emit_load_t(k):
                # t loads run two rounds ahead of use: t heads the long
                # diff -> ACT -> scan chain, and pulling its DMA traffic into
                # the DMA-light early window flattens the mid-kernel peak
                c0, cs = starts[k], CHUNK_SIZES[k]
                for fb in range(NFB):
                    r0 = fb * PB
                    tt = ttp.tile([PB, cs + 1], f32, tag="tt")
                    if k == 0:
                        nc.sync.dma_start(tt[:, 1 : cs + 1], t_d[r0 : r0 + PB, 0:cs])
                    else:
                        nc.sync.dma_start(
                            tt[:, 0 : cs + 1], t_d[r0 : r0 + PB, c0 - 1 : c0 + cs]
                        )
                    tts[(k, fb)] = tt

            def emit_load_x(k):
                c0, cs = starts[k], CHUNK_SIZES[k]
                for fb in range(NFB):
                    r0 = fb * PB
                    tx = txp.tile([PB, cs], f32, tag="tx")
                    nc.sync.dma_start(tx[:], x_d[r0 : r0 + PB, c0 : c0 + cs])
                    txs[(k, fb)] = tx

            def emit_decay(k, fbs=None):
                """diff on GPSIMD, then the 3-op ACT decay chain
                (dt -> ln -> exp -> exp = decay) written into PSUM."""
                c0, cs = starts[k], CHUNK_SIZES[k]
                for fb in (range(NFB) if fbs is None else fbs):
                    r0 = fb * PB
                    tt = tts.pop((k, fb))
                    dt = dtp.tile([PB, cs], f32, tag="dt")
                    if k == 0:
                        nc.gpsimd.tensor_tensor(
                            dt[:, 1:cs], tt[:, 2 : cs + 1], tt[:, 1:cs], op=Op.subtract
                        )
                        nc.gpsimd.memset(dt[:, 0:1], 1.0)
                    else:
                        nc.gpsimd.tensor_tensor(
                            dt[:, 0:cs], tt[:, 1 : cs + 1], tt[:, 0:cs], op=Op.subtract
                        )
                    # decay = exp(log_alpha * (dt*inv_mean)^power). Computed
                    # into PSUM: the scans then read data0 via the PSUM port,
                    # freeing an SBUF read port (GPSIMD shares SBUF ports with
                    # the DVE and was slowing the scans down by ~1.7x).
                    d = dpsp.tile([PB, cs], f32, tag="dps")
                    nc.scalar.activation(d[:], dt[:], Act.Ln, scale=im[fb][:])
                    nc.scalar.activation(d[:], d[:], Act.Exp, scale=pc[fb][:])
                    nc.scalar.activation(d[:], d[:], Act.Exp, scale=la[fb][:])
                    dts[(k, fb)] = d

            def emit_scan_and_out(k, fbs=None):
                c0, cs = starts[k], CHUNK_SIZES[k]
                for fb in (range(NFB) if fbs is None else fbs):
                    r0 = fb * PB
                    d = dts.pop((k, fb))
                    tx = txs.pop((k, fb))

                    # tc scan first: its consumer chain (ln/exp/mult) is longer
                    tcs = scanp.tile([PB, cs], f32, tag="tc")
                    init_tc = 0.0 if k == 0 else tcs_prev[fb]
                    nc.vector.tensor_tensor_scan(
                        tcs[:], d[:], ones[:, 0:cs], init_tc, op0=Op.mult, op1=Op.add
                    )
                    ws = scanp.tile([PB, cs], f32, tag="ws")
                    init_ws = 0.0 if k == 0 else ws_prev[fb]
                    nc.vector.tensor_tensor_scan(
                        ws[:], d[:], tx[:], init_ws, op0=Op.mult, op1=Op.add
                    )
                    ws_prev[fb] = ws[:, cs - 1 : cs]
                    tcs_prev[fb] = tcs[:, cs - 1 : cs]

                    # out = ws / tc via 1/tc = exp(-ln(tc)); tc >= 1, and both
                    # Ln and Exp live in the same ACT table set (no reloads)
                    L = lrp.tile([PB, cs], f32, tag="lr")
                    nc.scalar.activation(L[:], tcs[:], Act.Ln)
                    nc.scalar.activation(L[:], L[:], Act.Exp, scale=-1.0)
                    o = opool.tile([PB, cs], f32, tag="o")
                    # all multiplies on GPSIMD except the final round (DVE,
                    # to shorten the kernel tail)
                    if k == K - 1:
                        nc.vector.tensor_tensor(o[:], ws[:], L[:], op=Op.mult)
                    else:
                        nc.gpsimd.tensor_tensor(o[:], ws[:], L[:], op=Op.mult)
                    nc.sync.dma_start(out_d[r0 : r0 + PB, c0 : c0 + cs], o[:])

            emit_load_t(0)
            emit_load_x(0)
            emit_load_t(1)
            emit_params()
            emit_decay(0)
            for k in range(K):
                # per-fb interleave: the in-order ACT engine gets round-k
                # division work between the DMA/diff-gated round-k+1 decay
                # chains instead of stalling on them
                if k + 1 < K:
                    emit_load_x(k + 1)
                    emit_decay(k + 1, fbs=[0])
                    emit_scan_and_out(k, fbs=[0])
                    emit_decay(k + 1, fbs=[1])
                    emit_scan_and_out(k, fbs=[1])
                    if k + 2 < K:
                        # t prefetch rides at the END of the round's DMA queue
                        # so a tile-buffer stall cannot delay urgent transfers
                        emit_load_t(k + 2)
                else:
                    emit_scan_and_out(k)

    _split_multi_waits(nc, mybir)
    return nc


# Set by a profiling harness (test.py) to capture an NTFF profile of the run.
PROFILE_DIR = None
PROFILE_CORES = [0]


def kernel(x, t, log_halflife, power, running_sum_dt, dt_count):
    global _CACHED_NC
    from concourse.bass_utils import run_bass_kernel_spmd

    if _CACHED_NC is None:
        _CACHED_NC = _build_nc()
    nc = _CACHED_NC

    x = np.ascontiguousarray(np.asarray(x, dtype=np.float32))
    t = np.ascontiguousarray(np.asarray(t, dtype=np.float32))
    # O(F) parameter prep (mirrors the reference's scalar param math)
    lh = np.asarray(log_halflife, dtype=np.float32).reshape(F)
    pw = np.asarray(power, dtype=np.float32).reshape(F)
    rs = np.asarray(running_sum_dt, dtype=np.float32).reshape(F)
    cnt = np.float32(np.asarray(dt_count))
    la = (-np.float32(LN2) * np.exp(-lh)).astype(np.float32)
    pc = np.clip(pw, np.float32(1e-3), np.float32(1.0 - 1e-3)).astype(np.float32)
    im = (cnt / rs).astype(np.float32)
    prm = np.zeros((F, 4), dtype=np.float32)
    prm[:, 0], prm[:, 1], prm[:, 2] = la, pc, im
    prm = np.ascontiguousarray(prm)

    in_maps = [{"x": x[b], "t": t[b], "params": prm} for b in range(NCORES)]

    if PROFILE_DIR is not None:
        import os
        from trn_agent_boot.trn_boot import _ntff_profile_via_ctypes

        os.makedirs(PROFILE_DIR, exist_ok=True)
        hook = _ntff_profile_via_ctypes("/opt/axon/libaxon_pjrt.so")
        with hook(PROFILE_DIR, list(PROFILE_CORES)):
            res = run_bass_kernel_spmd(nc, in_maps, list(range(NCORES)))
    else:
        res = run_bass_kernel_spmd(nc, in_maps, list(range(NCORES)))

    out = np.stack([res.results[b]["out"] for b in range(NCORES)], axis=0)
    return out.astype(np.float32, copy=False)

